# revision 4
# baseline (speedup 1.0000x reference)
"""Trainium2 Bass kernel for nn_ChamferDistance (retrieval_knn).

Computes, for fixed shapes
    point   [128, 32, 2048, 3] f32
    CP      [128, 32, 32, 32, 3] f32
    tsdfOut [128, 65536] f32
    tsdfGT  [128, 65536] f32
    inUse   [128, 32] i32
the scalar
    mean(||pts - where(mask, CP[b, qx, qy, qz], pts)||) + mean(|sqrt(tsdfOut) - tsdfGT|)
with qk = clip(int((pts_k + 0.5) * 32), 0, 31).

Sharding: data-parallel over batch, 16 batches per NeuronCore across 8 cores.
Each core streams its `point`/tsdf slices, quantizes indices on DVE/ACT
(exact floor via the min/mod trick), gathers closest points straight from
DRAM with one 65536-index indirect DMA per batch, and reduces to per-partition
partial sums. Host sums the 8x[128,2] partials and forms the final scalar.
"""

import numpy as np

import concourse.bacc as bacc
import concourse.mybir as mybir
import concourse.tile as tile
from concourse import bass_utils
from concourse.bass import AP, IndirectOffsetOnAxis

GRID = 32
B, NP, NS = 128, 32, 2048
N = NP * NS            # 65536 samples per batch
P = 128                # SBUF partitions
NCORES = 8
NB = B // NCORES       # 16 batches per core
M = N // P             # 512 samples per partition per batch
CELLS = GRID**3        # 32768

_cache: dict = {}


def _build_module():
    f32 = mybir.dt.float32
    i32 = mybir.dt.int32
    AF = mybir.ActivationFunctionType
    ALU = mybir.AluOpType
    AX = mybir.AxisListType

    nc = bacc.Bacc("TRN2", debug=False, enable_asserts=False, num_devices=NCORES)

    point = nc.dram_tensor("point", [NB, N, 3], f32, kind="ExternalInput")
    cp = nc.dram_tensor("cp", [NB * CELLS, 3], f32, kind="ExternalInput")
    tsdf_out = nc.dram_tensor("tsdf_out", [NB, N], f32, kind="ExternalInput")
    tsdf_gt = nc.dram_tensor("tsdf_gt", [NB, N], f32, kind="ExternalInput")
    in_use = nc.dram_tensor("in_use", [NB, NP], i32, kind="ExternalInput")
    out = nc.dram_tensor("out", [P, 2], f32, kind="ExternalOutput")

    # const AP for activation bias=16.0 (mirrors Bass.__init__'s registration)
    t16 = nc.alloc_sbuf_tensor("const-f32-16", [P, 1], f32)
    nc.gpsimd.memset(t16.ap(), 16.0)
    nc.const_aps.aps[(f32, 16.0)] = t16.ap()
    nc.all_engine_barrier()

    with tile.TileContext(nc) as tc:
        with (
            tc.tile_pool(name="big", bufs=3) as big_pool,
            tc.tile_pool(name="small", bufs=3) as small_pool,
            tc.tile_pool(name="acc", bufs=1) as acc_pool,
        ):
            acc = acc_pool.tile([P, 2], f32)
            nc.vector.memset(acc[:], 0.0)

            for b in range(NB):
                # ---- load point tile: [P, M*3]; partition p <- samples [p*M,(p+1)*M)
                pts = big_pool.tile([P, M * 3], f32, tag="pts")
                nc.sync.dma_start(
                    out=pts[:], in_=point[b].rearrange("(p m) c -> p (m c)", p=P)
                )
                pts3 = pts[:].rearrange("p (m c) -> p m c", c=3)

                # ---- quantize: u = relu(32*v + 16); q = floor(min(u, 31.5))
                u = big_pool.tile([P, M * 3], f32, tag="u")
                u3 = u[:].rearrange("p (m c) -> p m c", c=3)
                for c in range(3):
                    nc.scalar.activation(
                        out=u3[:, :, c], in_=pts3[:, :, c], func=AF.Relu,
                        bias=16.0, scale=32.0,
                    )
                # floor via round-to-nearest (magic 2^23 add) + is_gt fixup:
                #   cc = min(u, 31.5); rf = rne(cc); q = rf - (rf > cc)
                MAGIC = 8388608.0  # 2^23
                cc = big_pool.tile([P, M * 3], f32, tag="cc")
                nc.vector.tensor_scalar(
                    out=cc[:], in0=u[:], scalar1=31.5, scalar2=None, op0=ALU.min,
                )
                rf = big_pool.tile([P, M * 3], f32, tag="rf")
                nc.vector.tensor_scalar(
                    out=rf[:], in0=cc[:], scalar1=MAGIC, scalar2=-MAGIC,
                    op0=ALU.add, op1=ALU.add,
                )
                gt = big_pool.tile([P, M * 3], f32, tag="gt")
                nc.vector.tensor_tensor(
                    out=gt[:], in0=rf[:], in1=cc[:], op=ALU.is_gt,
                )
                q = big_pool.tile([P, M * 3], f32, tag="q")
                nc.vector.tensor_tensor(
                    out=q[:], in0=rf[:], in1=gt[:], op=ALU.subtract,
                )
                q3 = q[:].rearrange("p (m c) -> p m c", c=3)

                # flat = qx*1024 + (qy*32 + qz) + b*32768, exact in f32
                t1 = small_pool.tile([P, M], f32, tag="t1")
                nc.vector.scalar_tensor_tensor(
                    out=t1[:], in0=q3[:, :, 1], scalar=32.0, in1=q3[:, :, 2],
                    op0=ALU.mult, op1=ALU.add,
                )
                flatf = small_pool.tile([P, M], f32, tag="flatf")
                nc.vector.scalar_tensor_tensor(
                    out=flatf[:], in0=q3[:, :, 0], scalar=1024.0, in1=t1[:],
                    op0=ALU.mult, op1=ALU.add,
                )
                idx = small_pool.tile([P, M], i32, tag="idx")
                nc.vector.tensor_scalar(
                    out=idx[:], in0=flatf[:], scalar1=float(b * CELLS),
                    scalar2=None, op0=ALU.add,
                )

                # ---- gather: g[p, 3m:3m+3] = cp[idx[p, m], :]
                g = big_pool.tile([P, M * 3], f32, tag="g")
                nc.gpsimd.indirect_dma_start(
                    out=g[:], out_offset=None,
                    in_=cp[:], in_offset=IndirectOffsetOnAxis(ap=idx[:], axis=0),
                )

                # ---- dist = sqrt(sum_c (pts-g)^2); per-partition sum via accum
                diff = big_pool.tile([P, M * 3], f32, tag="diff")
                nc.vector.tensor_tensor(
                    out=diff[:], in0=pts[:], in1=g[:], op=ALU.subtract
                )
                nc.vector.tensor_tensor(
                    out=diff[:], in0=diff[:], in1=diff[:], op=ALU.mult
                )
                d2 = small_pool.tile([P, M], f32, tag="d2")
                nc.vector.tensor_reduce(
                    out=d2[:], in_=diff[:].rearrange("p (m c) -> p m c", c=3),
                    axis=AX.X, op=ALU.add,
                )
                dist = small_pool.tile([P, M], f32, tag="dist")
                dsum = small_pool.tile([P, 1], f32, tag="dsum")
                nc.scalar.activation(
                    out=dist[:], in_=d2[:], func=AF.Sqrt, accum_out=dsum[:]
                )

                # ---- inUse mask: sample (p, m) belongs to primitive p//4
                mask_i = small_pool.tile([P, 1], i32, tag="mask_i")
                nc.sync.dma_start(
                    out=mask_i[:], in_=AP(in_use, b * NP, [[1, NP], [0, P // NP]])
                )
                maskf = small_pool.tile([P, 1], f32, tag="maskf")
                nc.vector.tensor_scalar(
                    out=maskf[:], in0=mask_i[:], scalar1=1, scalar2=None,
                    op0=ALU.is_equal,
                )
                nc.vector.scalar_tensor_tensor(
                    out=acc[:, 0:1], in0=dsum[:], scalar=maskf[:], in1=acc[:, 0:1],
                    op0=ALU.mult, op1=ALU.add,
                )

                # ---- tsdf: sum |sqrt(tsdfOut) - tsdfGT|
                to_t = small_pool.tile([P, M], f32, tag="to_t")
                tg_t = small_pool.tile([P, M], f32, tag="tg_t")
                nc.sync.dma_start(
                    out=to_t[:], in_=tsdf_out[b].rearrange("(p m) -> p m", p=P)
                )
                nc.sync.dma_start(
                    out=tg_t[:], in_=tsdf_gt[b].rearrange("(p m) -> p m", p=P)
                )
                sq = small_pool.tile([P, M], f32, tag="sq")
                nc.scalar.activation(out=sq[:], in_=to_t[:], func=AF.Sqrt)
                nc.vector.tensor_tensor(
                    out=sq[:], in0=sq[:], in1=tg_t[:], op=ALU.subtract
                )
                tsum = small_pool.tile([P, 1], f32, tag="tsum")
                nc.vector.tensor_reduce(
                    out=tsum[:], in_=sq[:], axis=AX.X, op=ALU.add,
                    apply_absolute_value=True,
                )
                nc.vector.tensor_tensor(
                    out=acc[:, 1:2], in0=acc[:, 1:2], in1=tsum[:], op=ALU.add
                )

            nc.sync.dma_start(out=out[:], in_=acc[:])

    nc.compile()
    return nc


def _make_in_maps(point, CP, tsdfOut, tsdfGT, inUse):
    point = np.ascontiguousarray(point, dtype=np.float32).reshape(B, N, 3)
    CP = np.ascontiguousarray(CP, dtype=np.float32).reshape(B, CELLS, 3)
    tsdfOut = np.ascontiguousarray(tsdfOut, dtype=np.float32)
    tsdfGT = np.ascontiguousarray(tsdfGT, dtype=np.float32)
    inUse = np.ascontiguousarray(inUse, dtype=np.int32)
    in_maps = []
    for c in range(NCORES):
        s = slice(c * NB, (c + 1) * NB)
        in_maps.append({
            "point": point[s],
            "cp": CP[s].reshape(NB * CELLS, 3),
            "tsdf_out": tsdfOut[s],
            "tsdf_gt": tsdfGT[s],
            "in_use": inUse[s],
        })
    return in_maps


def get_module():
    if "nc" not in _cache:
        _cache["nc"] = _build_module()
    return _cache["nc"]


def kernel(point, CP, tsdfOut, tsdfGT, inUse):
    nc = get_module()
    in_maps = _make_in_maps(point, CP, tsdfOut, tsdfGT, inUse)
    res = bass_utils.run_bass_kernel_spmd(nc, in_maps, core_ids=list(range(NCORES)))
    parts = np.stack([r["out"] for r in res.results])  # [8, 128, 2]
    sums = parts.sum(axis=(0, 1), dtype=np.float64)
    total = (sums[0] + sums[1]) / float(B * N)
    return np.float32(total)
